# revision 46
# baseline (speedup 1.0000x reference)
"""DisparityWarp Trainium2 kernel (Bass/Tile) — v2.

Contract: kernel(src, disparity) takes FULL inputs
  src [8, 32, 384, 768] f32, disparity [8, 1, 384, 768] f32
and returns the FULL output [8, 32, 384, 768] f32 of
  grid_sample(src, grid, bilinear, zeros, align_corners=False)
with grid_x = 2*(xx - disp)/(W-1) - 1, grid_y = 2*yy/(H-1) - 1.

Sharding: pure data parallel, one batch per NeuronCore (8 cores).

Per-core algorithm: unnormalized coords ix = (x - d)*W/(W-1) - 0.5,
iy = y*H/(H-1) - 0.5. Vertical lerp weights depend only on y; the
horizontal warp is banded: out[c,x] = sum_x' vrow[c,x'] * hat(ix[x]-x')
with hat(u) = max(0, 1-|u|), and x' restricted to [0, W) (zero pad).

Geometry: output cols in blocks of BLK=94; window j covers
x' in [lo_j, hi_j), lo_j = max(94j-33, 0), hi_j = min(94j+95, W).
Windows align at partition 0, so no zero-padding of the source is
needed: window 0 starts at x'=0 and window 8 is clipped to 49 columns
(out-of-range x' simply never enters the contraction = zero padding).

Per 3-row group g (quad base qb = clamp(floor(iy[3g]), 0, H-4)):
  s4   [128=(4r,32c), W] f16  <- gpsimd casting DMA of src[:, qb:qb+4, :]
  VT:  vtp[x'loc, (3i,32c)] = s4[:, lo_j:hi_j].T @ v2neg[g]  (9 matmuls)
  vtsb f16 <- vtp  (PSUM evac split between ACT and DVE)
  per row i: D0 = ld2.T @ [int;frac] = ixm1[x] on all 128 partitions
     asb = |D0 - (p+1)|      (ACT activation Abs with bias, or DVE)
     wsb = min(asb - 1, 0)   (DVE / Pool)  == -hat
  gathers: outp[32i:+32, blk_j] = vtsb[:, 96j+32i:+32].T @ wsb[:, blk_j]
     (tile_position (0,32i); window 8 contracts K=49 only)
  outsb [96, W] f32 <- outp (ACT+DVE); DMA to out[:, 3g:3g+3, :].

The PE stream runs gathers one group behind VT/D0 so the weight chain
(ACT/DVE/Pool) hides behind the next group's PE work.
"""

import sys

if "/opt/trn_rl_repo" not in sys.path:
    sys.path.insert(0, "/opt/trn_rl_repo")

from contextlib import ExitStack

import numpy as np

import concourse.bass as bass
import concourse.mybir as mybir
from concourse import bacc
from concourse.tile import TileContext

F32 = mybir.dt.float32
F16 = mybir.dt.float16
I32 = mybir.dt.int32
AF = mybir.ActivationFunctionType
ALU = mybir.AluOpType

B, C, H, W = 8, 32, 384, 768
S = W / (W - 1)
BLK = 94           # output columns per block
NB = 9             # ceil(W / BLK)
GRP = 3            # output rows per group
NG = H // GRP      # 128 groups
N_CORES = 8

WIN_LO = [max(BLK * j - 33, 0) for j in range(NB)]
WIN_HI = [min(lo + 128, W) for lo in WIN_LO]
# vtp / outp PSUM column offsets (bank-aligned: banks of 512 f32)
VCOL = [96 * j if j < 5 else 512 + 96 * (j - 5) for j in range(NB)]
OCOL = [BLK * j if j < 5 else 512 + BLK * (j - 5) for j in range(NB)]


# ---------------------------------------------------------------- constants
def _vert_coefs():
    yy = np.arange(H, dtype=np.float64)
    iy = yy * (H / (H - 1)) - 0.5
    y0 = np.floor(iy).astype(np.int64)
    fy = iy - y0
    a = (1.0 - fy) * ((y0 >= 0) & (y0 < H))
    b = fy * ((y0 + 1 >= 0) & (y0 + 1 < H))
    return a, b, y0


def _host_constants():
    a, b, y0 = _vert_coefs()
    # V2NEG [128 p=(4r,32c), NG, 96 m=(3i,32c)] fp16, negated blend coefs
    v2 = np.zeros((4, C, NG, GRP, C), dtype=np.float32)
    quad_bases = []
    for g in range(NG):
        ys = [GRP * g + i for i in range(GRP)]
        qbase = min(max(int(y0[ys[0]]), 0), H - 4)
        quad_bases.append(qbase)
        for i, y in enumerate(ys):
            ra = int(y0[y]) - qbase
            rb = ra + 1
            for c in range(C):
                if a[y] != 0.0:
                    assert 0 <= ra <= 3
                    v2[ra, c, g, i, c] += -a[y]
                if b[y] != 0.0:
                    assert 0 <= rb <= 3
                    v2[rb, c, g, i, c] += -b[y]
    v2neg = v2.reshape(4 * C, NG, GRP * C).astype(np.float16)

    # D matmul stationary: rows [-(p+1), 1, 1]; rhs rows [ones, int, frac]
    ld3 = np.stack([
        -(np.arange(128, dtype=np.float32) + 1.0),
        np.ones(128, dtype=np.float32),
        np.ones(128, dtype=np.float32),
    ]).astype(np.float16)                                        # [3, 128]
    onesw = np.ones((1, 2 * GRP, W), dtype=np.float16)           # [1, 6, W]

    x = np.arange(W, dtype=np.float64)
    base = np.array([WIN_LO[int(xi) // BLK] for xi in x], dtype=np.float64)
    cf = (x * S - 0.5 - base + 1.0).astype(np.float32)[None, :]  # [1, W]
    return v2neg, ld3, onesw, cf, quad_bases


# ---------------------------------------------------------------- program
def build_nc(ngroups=NG):
    _, _, _, _, quad_bases = _host_constants()  # (v2neg, ld3, onesw, cf, qb)
    nc = bacc.Bacc("TRN2", target_bir_lowering=False, debug=False)

    src = nc.dram_tensor("src", [C, H, W], F32, kind="ExternalInput").ap()
    disp = nc.dram_tensor("disp", [H, W], F32, kind="ExternalInput").ap()
    v2d = nc.dram_tensor("v2neg", [4 * C, NG, GRP * C], F16,
                         kind="ExternalInput").ap()
    ld3d = nc.dram_tensor("ld3", [3, 128], F16, kind="ExternalInput").ap()
    onesd = nc.dram_tensor("onesw", [1, 2 * GRP, W], F16,
                           kind="ExternalInput").ap()
    cfd = nc.dram_tensor("cf", [1, W], F32, kind="ExternalInput").ap()
    outd = nc.dram_tensor("out", [C, H, W], F32, kind="ExternalOutput").ap()

    ngr = min(ngroups, NG)
    nrows = GRP * ngr
    nt = (nrows + 127) // 128

    with ExitStack() as ctx:
        tc = ctx.enter_context(TileContext(nc))
        singles = ctx.enter_context(tc.tile_pool(name="singles", bufs=1))
        ph1 = ctx.enter_context(tc.tile_pool(name="ph1", bufs=2))
        s4p = ctx.enter_context(tc.tile_pool(name="s4p", bufs=4))
        vtap = ctx.enter_context(tc.tile_pool(name="vtap", bufs=2))
        vtbp = ctx.enter_context(tc.tile_pool(name="vtbp", bufs=2))
        asbp = ctx.enter_context(tc.tile_pool(name="asbp", bufs=3))
        wp = ctx.enter_context(tc.tile_pool(name="wp", bufs=6))
        obufp = ctx.enter_context(tc.tile_pool(name="obufp", bufs=2))
        vtpp = ctx.enter_context(tc.tile_pool(name="vtpp", bufs=1, space="PSUM"))
        dpp = ctx.enter_context(tc.tile_pool(name="dpp", bufs=2, space="PSUM"))
        outpp = ctx.enter_context(tc.tile_pool(name="outpp", bufs=1, space="PSUM"))

        # ---- constants ----
        v2sb = singles.tile([4 * C, NG, GRP * C], F16)
        nc.sync.dma_start(out=v2sb, in_=v2d)
        ld3sb = singles.tile([3, 128], F16)
        nc.sync.dma_start(out=ld3sb, in_=ld3d)
        cfb = singles.tile([128, W], F32)
        nc.sync.dma_start(out=cfb, in_=cfd.to_broadcast((128, W)))

        # ---- persistent slab ring (ones partition written once) ----
        NSLAB = 4
        slabring = [singles.tile([3, 2 * GRP, W], F16, name=f"slabr{k}",
                                 tag=f"slabr{k}")
                    for k in range(NSLAB)]
        for t_ in slabring:
            nc.vector.memset(t_[0:1, :, :], 1.0)

        # ---- phase 1: disparity -> int/frac fp16 [128, nt, W] ----
        int16 = singles.tile([128, nt, W], F16)
        frac16 = singles.tile([128, nt, W], F16)
        for t in range(nt):
            r0 = 128 * t
            nr = min(128, H - r0)
            dt_ = ph1.tile([128, W], F32)
            nc.sync.dma_start(out=dt_[:nr], in_=disp[r0:r0 + nr, :])
            ixm1 = ph1.tile([128, W], F32)
            nc.vector.tensor_scalar_mul(ixm1[:nr], dt_[:nr], -float(S))
            nc.vector.tensor_add(ixm1[:nr], ixm1[:nr], cfb[:nr])
            iv = ph1.tile([128, W], I32)
            nc.vector.tensor_copy(iv[:nr], ixm1[:nr])
            fv = ph1.tile([128, W], F32)
            nc.vector.tensor_copy(fv[:nr], iv[:nr])
            nc.scalar.copy(int16[:nr, t, :], fv[:nr])
            fr = ph1.tile([128, W], F32)
            nc.vector.tensor_sub(fr[:nr], ixm1[:nr], fv[:nr])
            nc.scalar.copy(frac16[:nr, t, :], fr[:nr])

        # ---- staging helpers ----
        def issue_s4(g):
            qb = quad_bases[g]
            s4 = s4p.tile([128, W], F16, name=f"s4_{g}", tag="s4")
            in_ap = src[:, qb:qb + 4, :].rearrange("c r x -> r c x")
            nc.gpsimd.dma_start(out=s4, in_=in_ap)
            return s4

        def issue_slab(pair):
            # One slab serves groups (2*pair, 2*pair+1): rows 6*pair..+5.
            # slab partitions: 0 = ones (persistent), 1 = int, 2 = frac.
            y0 = 2 * GRP * pair
            cnt = min(2 * GRP, nrows - y0)
            slab = slabring[pair % NSLAB]
            for part, tsrc in ((1, int16), (2, frac16)):
                done = 0
                while done < cnt:
                    y = y0 + done
                    p, t = y % 128, y // 128
                    n = min(cnt - done, 128 - p)
                    nc.sync.dma_start(
                        out=slab[part:part + 1, done:done + n, :],
                        in_=tsrc[p:p + n, t, :])
                    done += n
            return slab

        def gathers(rec, i, outp):
            wsb = rec["wsb"][i]
            vta, vtb = rec["vta"], rec["vtb"]
            for j in range(NB):
                n = min(BLK, W - BLK * j)
                if j < 5:
                    lhs = vta[:, 96 * j + 32 * i:96 * j + 32 * i + 32]
                elif j < 8:
                    lhs = vtb[:, 96 * (j - 5) + 32 * i:96 * (j - 5) + 32 * i + 32]
                else:
                    lhs = vtb[0:49, 288 + 32 * i:288 + 32 * i + 32]
                rhs = (wsb[:, BLK * j:BLK * j + n] if j < 8
                       else wsb[0:49, BLK * j:BLK * j + n])
                nc.tensor.matmul(
                    outp[32 * i:32 * i + 32, OCOL[j]:OCOL[j] + n],
                    lhs, rhs, start=True, stop=True,
                    tile_position=(0, 32 * i),
                )

        # ---- prologue prefetch ----
        npair = (ngr + 1) // 2
        pre_s4 = {}
        pre_slab = {}
        for g in range(min(3, ngr)):
            pre_s4[g] = issue_s4(g)
        for p_ in range(min(3, npair)):
            pre_slab[p_] = issue_slab(p_)

        # Output stores are batched OBATCH groups per DMA: per-group PSUM
        # evac copies land in one wide SBUF tile; a single DMA (emitted
        # right after the batch's last copy, so its wait is ~resolved)
        # stores 3*OBATCH rows.  Few out DMAs -> the 8 round-robin DMAHW
        # completion lanes are never blocked by long-waiting stores, which
        # otherwise throttles the prefetch DMA stream behind them.
        OBATCH = 4
        obuf_state = {"tile": None, "base": -1}
        pending_out = []   # queued store DMAs, drained one per iteration

        def store_prev(prev, outp):
            pg = prev["g"]
            b = pg % OBATCH
            if b == 0:
                obuf_state["tile"] = obufp.tile([96, OBATCH, W], F32,
                                                name=f"obuf_{pg}",
                                                tag="obuf")
                obuf_state["base"] = pg
            obuf = obuf_state["tile"]
            nc.vector.tensor_copy(obuf[:, b, 0:470], outp[:, 0:470])
            nc.scalar.copy(obuf[:, b, 470:W], outp[:, 512:810])
            if b == OBATCH - 1 or pg == ngr - 1:
                g0 = obuf_state["base"]
                nb = pg - g0 + 1
                rows = outd[:, GRP * g0:GRP * (g0 + nb), :]
                for i in range(GRP):
                    pending_out.append(
                        (rows[:, i::GRP, :], obuf[32 * i:32 * i + 32, 0:nb, :]))

        def drain_out(all_=False):
            # One store DMA per iteration: data is >=1 batch old, so the
            # wait is resolved at issue and the SP queue never head-blocks.
            n = len(pending_out) if all_ else (1 if pending_out else 0)
            for _ in range(n):
                o, i_ = pending_out.pop(0)
                nc.sync.dma_start(out=o, in_=i_)

        prev = None
        for g in range(ngr):
            s4 = pre_s4.pop(g)
            slab = pre_slab[g // 2]
            if g % 2 == 1:
                del pre_slab[g // 2]
            iloc0 = GRP * (g % 2)

            # ---- prefetch first: these SP DMAs have no unresolved waits,
            # so they must sit AHEAD of the out DMAs in the SP queue ----
            if g + 3 < ngr:
                pre_s4[g + 3] = issue_s4(g + 3)
            if g % 2 == 0 and g // 2 + 3 < npair:
                pre_slab[g // 2 + 3] = issue_slab(g // 2 + 3)
            drain_out()

            # ---- VT: blend-transpose, 9 windows ----

            vtp = vtpp.tile([128, 1024], F32, name=f"vtp_{g}", tag="vtp")
            for j in range(NB):
                m = WIN_HI[j] - WIN_LO[j]
                nc.tensor.matmul(
                    vtp[0:m, VCOL[j]:VCOL[j] + GRP * C],
                    s4[:, WIN_LO[j]:WIN_HI[j]],
                    v2sb[:, g, :],
                    start=True, stop=True,
                )
            vta = vtap.tile([128, 480], F16, name=f"vta_{g}", tag="vta")
            nc.vector.tensor_copy(vta, vtp[:, 0:480])
            vtb = vtbp.tile([128, 384], F16, name=f"vtb_{g}", tag="vtb")
            nc.vector.tensor_copy(vtb[:, 0:288], vtp[:, 512:800])
            nc.vector.tensor_copy(vtb[0:49, 288:384], vtp[0:49, 800:896])

            # ---- D rows 0, 1 + weight chains ----
            dp0 = dpp.tile([128, 1024], F32, name=f"dp0_{g}", tag="dp")
            nc.tensor.matmul(dp0[:, 0:512], ld3sb,
                             slab[0:3, iloc0 + 0, 0:512],
                             start=True, stop=True)
            nc.tensor.matmul(dp0[:, 512:768], ld3sb,
                             slab[0:3, iloc0 + 0, 512:W],
                             start=True, stop=True)
            dp1 = dpp.tile([128, 1024], F32, name=f"dp1_{g}", tag="dp")
            nc.tensor.matmul(dp1[:, 0:512], ld3sb,
                             slab[0:3, iloc0 + 1, 0:512],
                             start=True, stop=True)
            nc.tensor.matmul(dp1[:, 512:768], ld3sb,
                             slab[0:3, iloc0 + 1, 512:W],
                             start=True, stop=True)

            asb0 = asbp.tile([128, W], F16, name=f"asb0_{g}", tag="asb")
            nc.scalar.activation(asb0, dp0[:, 0:W], AF.Abs)
            wsb0 = wp.tile([128, W], F16, name=f"wsb0_{g}", tag="wsb")
            nc.vector.tensor_scalar(out=wsb0, in0=asb0, scalar1=1.0,
                                    scalar2=0.0, op0=ALU.subtract,
                                    op1=ALU.min)
            asb1 = asbp.tile([128, W], F16, name=f"asb1_{g}", tag="asb")
            nc.scalar.activation(asb1, dp1[:, 0:W], AF.Abs)
            wsb1 = wp.tile([128, W], F16, name=f"wsb1_{g}", tag="wsb")
            nc.vector.tensor_scalar(out=wsb1, in0=asb1, scalar1=1.0,
                                    scalar2=0.0, op0=ALU.subtract,
                                    op1=ALU.min)

            # ---- gathers for previous group, row 0 ----
            outp = None
            if prev is not None:
                outp = outpp.tile([96, 1024], F32, name=f"outp_{g}",
                                  tag="outp")
                gathers(prev, 0, outp)

            # ---- D row 2 + weight chain (DVE abs) ----
            dp2 = dpp.tile([128, 1024], F32, name=f"dp2_{g}", tag="dp")
            nc.tensor.matmul(dp2[:, 0:512], ld3sb,
                             slab[0:3, iloc0 + 2, 0:512],
                             start=True, stop=True)
            nc.tensor.matmul(dp2[:, 512:768], ld3sb,
                             slab[0:3, iloc0 + 2, 512:W],
                             start=True, stop=True)
            asb2 = asbp.tile([128, W], F16, name=f"asb2_{g}", tag="asb")
            nc.scalar.activation(asb2, dp2[:, 0:W], AF.Abs)
            wsb2 = wp.tile([128, W], F16, name=f"wsb2_{g}", tag="wsb")
            nc.vector.tensor_scalar(out=wsb2, in0=asb2, scalar1=1.0,
                                    scalar2=0.0, op0=ALU.subtract,
                                    op1=ALU.min)

            if prev is not None:
                gathers(prev, 1, outp)
                gathers(prev, 2, outp)
                store_prev(prev, outp)

            prev = {"g": g, "wsb": (wsb0, wsb1, wsb2), "vta": vta,
                    "vtb": vtb}

        # ---- epilogue: drain last group ----
        outp = outpp.tile([96, 1024], F32, name="outp_last", tag="outp")
        for i in range(GRP):
            gathers(prev, i, outp)
        store_prev(prev, outp)
        drain_out(all_=True)

    nc.finalize()
    return nc


_NC_CACHE = {}


def _get_nc(ngroups=NG):
    if ngroups not in _NC_CACHE:
        _NC_CACHE[ngroups] = build_nc(ngroups)
    return _NC_CACHE[ngroups]


# ---------------------------------------------------------------- entry
def kernel(src: np.ndarray, disparity: np.ndarray) -> np.ndarray:
    from concourse.bass_utils import run_bass_kernel_spmd

    src = np.ascontiguousarray(np.asarray(src), dtype=np.float32)
    disparity = np.ascontiguousarray(np.asarray(disparity), dtype=np.float32)
    v2neg, ld3, onesw, cf, _ = _host_constants()
    nc = _get_nc()
    in_maps = []
    for b in range(B):
        in_maps.append({
            "src": src[b],
            "disp": disparity[b, 0],
            "v2neg": v2neg,
            "ld3": ld3,
            "onesw": onesw,
            "cf": cf,
        })
    res = run_bass_kernel_spmd(nc, in_maps, core_ids=list(range(N_CORES)))
    out = np.stack([res.results[b]["out"] for b in range(B)])
    return out.astype(np.float32)


# ---------------------------------------------------------------- sim test
def _np_reference(src, disp):
    """Single-core numpy reference (mirror of reference.py)."""
    Cc, Hh, Ww = src.shape
    xx = np.arange(Ww, dtype=np.float32)
    ix = (xx[None, :] - disp) * (Ww / (Ww - 1)) - 0.5          # [H, W]
    yy = np.arange(Hh, dtype=np.float32)
    iy = np.broadcast_to((yy * (Hh / (Hh - 1)) - 0.5)[:, None], (Hh, Ww))
    x0 = np.floor(ix).astype(np.int64)
    y0 = np.floor(iy).astype(np.int64)
    fx = ix - x0
    fy = iy - y0

    def gather(yi, xi):
        inb = ((yi >= 0) & (yi < Hh) & (xi >= 0) & (xi < Ww))
        yc = np.clip(yi, 0, Hh - 1)
        xc = np.clip(xi, 0, Ww - 1)
        v = src[:, yc, xc]                                      # [C, H, W]
        return v * inb[None]

    w00 = (1 - fy) * (1 - fx)
    w01 = (1 - fy) * fx
    w10 = fy * (1 - fx)
    w11 = fy * fx
    return (gather(y0, x0) * w00 + gather(y0, x0 + 1) * w01 +
            gather(y0 + 1, x0) * w10 + gather(y0 + 1, x0 + 1) * w11)


def _sim_check(ngroups=2):
    from concourse.bass_interp import CoreSim

    rng = np.random.default_rng(0)
    src = rng.standard_normal((C, H, W)).astype(np.float32)
    disp = (rng.random((H, W)) * 32.0).astype(np.float32)
    v2neg, ld3, onesw, cf, _ = _host_constants()

    nc = build_nc(ngroups)
    sim = CoreSim(nc)
    for name, val in (("src", src), ("disp", disp), ("v2neg", v2neg),
                      ("ld3", ld3), ("onesw", onesw), ("cf", cf)):
        sim.tensor(name)[:] = val
    sim.simulate(check_with_hw=False)
    got = np.array(sim.tensor("out"))

    ref = _np_reference(src, disp)
    ys = slice(0, GRP * ngroups)
    diff = got[:, ys] - ref[:, ys]
    rel = np.linalg.norm(diff) / np.linalg.norm(ref[:, ys])
    print(f"sim rows[0:{GRP * ngroups}]  max abs "
          f"{np.abs(diff).max():.3e}  rel l2 {rel:.3e}")
    return rel


if __name__ == "__main__":
    ng = int(sys.argv[1]) if len(sys.argv) > 1 else 2
    _sim_check(ng)
